# revision 1
# baseline (speedup 1.0000x reference)
"""GIN message-passing kernel for 8 TRN2 NeuronCores.

Strategy (per sharding hint): nodes are sharded across 8 cores (6272 slots
each, 50176 padded total). Edges are partitioned by destination node so each
core's segment-sum is local; source-node features are gathered by row index
(dma_gather) from a replicated full table (x for layer 1, an AllGather'ed h1
for layer 2). MLP weights are replicated.

Per output tile of 128 nodes, in-edges (plus one self-edge per node) are
packed into 128-edge chunks. Each chunk is gathered as G [128 edges, 128
feats] and accumulated into a feature-major PSUM tile via
agg[f, i] += G.T @ M, where M[e, i] = (dst_off[e] == i) is a one-hot built
on-device with a single broadcast is_equal. dma_gather indices are int16, so
gathers are split at row 32768 (lo/hi base).
"""
import os
import warnings

warnings.filterwarnings("ignore")

import numpy as np

N = 50000
E = 800000
F = 128
H = 128
C = 40
BN_EPS = 1e-5
NCORES = 8
P = 128
NPC = 6272           # node slots per core
NT = NPC // P        # 49 tiles per core
NPAD = NCORES * NPC  # 50176
LOSPLIT = 32768      # int16 gather index limit


# ----------------------------------------------------------------- host prep

def _assign_nodes(deg):
    """Greedy balanced assignment of nodes to (core, tile, slot).

    Returns gid_of_orig[N]: global slot id = c*NPC + t*P + s, balancing total
    degree per core and per tile so per-tile chunk counts are uniform.
    """
    order = np.argsort(-deg, kind="stable")
    core_load = np.zeros(NCORES, np.int64)
    core_cnt = np.zeros(NCORES, np.int64)
    node_core = np.empty(N, np.int32)
    for n in order:
        c = -1
        best = None
        for cc in range(NCORES):
            if core_cnt[cc] >= NPC:
                continue
            if best is None or core_load[cc] < best:
                best = core_load[cc]
                c = cc
        node_core[n] = c
        core_load[c] += deg[n]
        core_cnt[c] += 1

    gid_of_orig = np.empty(N, np.int64)
    for c in range(NCORES):
        nodes = order[node_core[order] == c]
        tile_load = np.zeros(NT, np.int64)
        tile_cnt = np.zeros(NT, np.int64)
        tl = np.empty(len(nodes), np.int32)
        # nodes are already degree-sorted desc; greedy least-loaded tile
        for i, n in enumerate(nodes):
            avail = tile_cnt < P
            t = np.where(avail, tile_load, np.iinfo(np.int64).max).argmin()
            tl[i] = t
            tile_load[t] += deg[n]
            tile_cnt[t] += 1
        slot = np.zeros(NT, np.int64)
        for i, n in enumerate(nodes):
            t = tl[i]
            gid_of_orig[n] = c * NPC + t * P + slot[t]
            slot[t] += 1
    return gid_of_orig


def _wrap_idx(idx):
    """[n] int -> [128, n//16] int16: idx i at [i%16, i//16], replicated x8."""
    n = len(idx)
    w = np.asarray(idx, np.int16).reshape(n // 16, 16).T
    return np.tile(w, (8, 1))


def _pack_edges(src_gid, dst_gid, rowmap, rows_total):
    """Partition edges by (core, tile), split lo/hi by gathered row id, pad to
    128-multiples, and build per-core packed idx (int16 wrapped) and dstoff
    (f32) arrays plus the per-tile chunk-count lists.

    rowmap: maps src gid -> row index in the gather table (identity for x).
    Returns (CH_LO[t], CH_HI[t], idx_pack[c], off_pack[c]).
    """
    rows = rowmap[src_gid]
    core = dst_gid // NPC
    tile = (dst_gid % NPC) // P
    off = dst_gid % P
    is_lo = rows < LOSPLIT

    # bucket edges per (core, tile, half)
    lists = [[None] * NT for _ in range(NCORES)]
    key = (core * NT + tile).astype(np.int64)
    order = np.argsort(key, kind="stable")
    rows_s, off_s, lo_s, key_s = rows[order], off[order], is_lo[order], key[order]
    bounds = np.searchsorted(key_s, np.arange(NCORES * NT + 1))
    for c in range(NCORES):
        for t in range(NT):
            b0, b1 = bounds[c * NT + t], bounds[c * NT + t + 1]
            m = lo_s[b0:b1]
            lists[c][t] = (
                (rows_s[b0:b1][m], off_s[b0:b1][m]),
                (rows_s[b0:b1][~m] - LOSPLIT, off_s[b0:b1][~m]),
            )

    CH_LO = np.zeros(NT, np.int64)
    CH_HI = np.zeros(NT, np.int64)
    for c in range(NCORES):
        for t in range(NT):
            lo, hi = lists[c][t]
            CH_LO[t] = max(CH_LO[t], -(-len(lo[0]) // P))
            CH_HI[t] = max(CH_HI[t], -(-len(hi[0]) // P))
    CH_LO = np.maximum(CH_LO, 1)
    CH_HI = np.maximum(CH_HI, 1)

    idx_pack, off_pack = [], []
    hi_rows = rows_total - LOSPLIT
    for c in range(NCORES):
        idx_cols, off_cols = [], []
        for t in range(NT):
            (lor, loo), (hir, hio) = lists[c][t]
            nlo, nhi = CH_LO[t] * P, CH_HI[t] * P
            li = np.zeros(nlo, np.int64)
            li[: len(lor)] = lor
            lf = np.full(nlo, P, np.float32)
            lf[: len(loo)] = loo
            hi_ = np.zeros(nhi, np.int64)
            hi_[: len(hir)] = hir
            hf = np.full(nhi, P, np.float32)
            hf[: len(hio)] = hio
            assert li.max(initial=0) < LOSPLIT and hi_.max(initial=0) < hi_rows
            idx_cols += [_wrap_idx(li), _wrap_idx(hi_)]
            # dstoff layout [128, CH]: chunk k, partition p = edge k*128+p
            off_cols += [
                lf.reshape(CH_LO[t], P).T.astype(np.float32),
                hf.reshape(CH_HI[t], P).T.astype(np.float32),
            ]
        idx_pack.append(np.ascontiguousarray(np.concatenate(idx_cols, axis=1)))
        off_pack.append(np.ascontiguousarray(np.concatenate(off_cols, axis=1)))
    return CH_LO, CH_HI, idx_pack, off_pack


def prepare(x, edge_index, W1a, bn_gamma, bn_beta, bn_mean, bn_var, W1b, W2a, W2b):
    x = np.asarray(x, np.float32)
    ei = np.asarray(edge_index, np.int64)
    src_o, dst_o = ei[0], ei[1]

    deg = np.bincount(dst_o, minlength=N).astype(np.int64) + 1  # + self edge
    gid_of_orig = _assign_nodes(deg)

    # self edges fold the "+h" term of GIN into the segment sum
    src_gid = np.concatenate([gid_of_orig[src_o], gid_of_orig])
    dst_gid = np.concatenate([gid_of_orig[dst_o], gid_of_orig])

    ident_map = np.arange(NPAD, dtype=np.int64)
    CH_LO, CH_HI, idx_pack, off_pack = _pack_edges(src_gid, dst_gid, ident_map, NPAD)

    x_pad = np.zeros((NPAD, F), np.float32)
    x_pad[gid_of_orig] = x

    scale = (np.asarray(bn_gamma) / np.sqrt(np.asarray(bn_var) + BN_EPS)).astype(
        np.float32
    )
    bias = (np.asarray(bn_beta) - np.asarray(bn_mean) * scale).astype(np.float32)

    consts = {
        "x_pad": x_pad,
        "W1aT": np.ascontiguousarray(np.asarray(W1a, np.float32).T),
        "W1bT": np.ascontiguousarray(np.asarray(W1b, np.float32).T),
        "W2aT": np.ascontiguousarray(np.asarray(W2a, np.float32).T),
        "W2bT": np.ascontiguousarray(np.asarray(W2b, np.float32).T),
        "bn_s": scale.reshape(H, 1),
        "bn_b": bias.reshape(H, 1),
        "iota": np.tile(np.arange(P, dtype=np.float32), (P, 1)),
    }
    in_maps = []
    for c in range(NCORES):
        m = dict(consts)
        m["idx_all"] = idx_pack[c]
        m["off_all"] = off_pack[c]
        in_maps.append(m)
    return in_maps, CH_LO, CH_HI, gid_of_orig


# -------------------------------------------------------------- bass program

def build(CH_LO, CH_HI, repeat=1, do_gather=True, do_compute=True, do_cc=True):
    import concourse.bacc as bacc
    import concourse.mybir as mybir
    import concourse.tile as tile
    from concourse.masks import make_identity

    nc = bacc.Bacc("TRN2", target_bir_lowering=False, debug=False, num_devices=NCORES)
    f32 = mybir.dt.float32

    S_TOT = int(8 * (CH_LO.sum() + CH_HI.sum()))
    CH_TOT = int(CH_LO.sum() + CH_HI.sum())
    CH_MAX = int((CH_LO + CH_HI).max())

    x_pad = nc.dram_tensor("x_pad", [NPAD, F], f32, kind="ExternalInput")
    idx_all = nc.dram_tensor("idx_all", [P, S_TOT], mybir.dt.int16, kind="ExternalInput")
    off_all = nc.dram_tensor("off_all", [P, CH_TOT], f32, kind="ExternalInput")
    W1aT = nc.dram_tensor("W1aT", [F, H], f32, kind="ExternalInput")
    W1bT = nc.dram_tensor("W1bT", [H, H], f32, kind="ExternalInput")
    W2aT = nc.dram_tensor("W2aT", [H, H], f32, kind="ExternalInput")
    W2bT = nc.dram_tensor("W2bT", [H, C], f32, kind="ExternalInput")
    bn_s = nc.dram_tensor("bn_s", [H, 1], f32, kind="ExternalInput")
    bn_b = nc.dram_tensor("bn_b", [H, 1], f32, kind="ExternalInput")
    iota = nc.dram_tensor("iota", [P, P], f32, kind="ExternalInput")
    outT = nc.dram_tensor("outT", [C, NPC], f32, kind="ExternalOutput")

    Relu = mybir.ActivationFunctionType.Relu

    with tile.TileContext(nc) as tc:
        with (
            tc.tile_pool(name="const", bufs=1) as cst,
            tc.tile_pool(name="gbuf", bufs=3) as gp,
            tc.tile_pool(name="mbuf", bufs=3) as mp,
            tc.tile_pool(name="small", bufs=3) as sp,
            tc.tile_pool(name="ps_agg", bufs=2, space="PSUM") as ps_agg,
            tc.tile_pool(name="ps_mm", bufs=2, space="PSUM") as ps_mm,
            tc.tile_pool(name="dram", bufs=1, space="DRAM") as dram,
        ):
            ident = cst.tile([P, P], f32)
            make_identity(nc, ident[:])
            iota_sb = cst.tile([P, P], f32)
            nc.sync.dma_start(out=iota_sb[:], in_=iota[:])
            w1a_sb = cst.tile([F, H], f32)
            nc.sync.dma_start(out=w1a_sb[:], in_=W1aT[:])
            w1b_sb = cst.tile([H, H], f32)
            nc.sync.dma_start(out=w1b_sb[:], in_=W1bT[:])
            w2a_sb = cst.tile([H, H], f32)
            nc.sync.dma_start(out=w2a_sb[:], in_=W2aT[:])
            w2b_sb = cst.tile([H, C], f32)
            nc.sync.dma_start(out=w2b_sb[:], in_=W2bT[:])
            bns_sb = cst.tile([H, 1], f32)
            nc.sync.dma_start(out=bns_sb[:], in_=bn_s[:])
            bnb_sb = cst.tile([H, 1], f32)
            nc.sync.dma_start(out=bnb_sb[:], in_=bn_b[:])
            idx_sb = cst.tile([P, S_TOT], mybir.dt.int16)
            nc.sync.dma_start(out=idx_sb[:], in_=idx_all[:])
            off_sb = cst.tile([P, CH_TOT], f32)
            nc.sync.dma_start(out=off_sb[:], in_=off_all[:])

            h1_slice = dram.tile([NPC, H], f32)
            h1_full = dram.tile([NPAD, H], f32)

            # column offsets per tile into idx_all / off_all
            icol = np.concatenate([[0], np.cumsum((CH_LO + CH_HI) * 8)])
            ocol = np.concatenate([[0], np.cumsum(CH_LO + CH_HI)])

            def aggregate(t, table):
                """Gather + segment-sum for tile t -> PSUM [F, P] feat-major."""
                chl, chh = int(CH_LO[t]), int(CH_HI[t])
                ch = chl + chh
                G = gp.tile([P, CH_MAX, F], f32, tag="G")
                ic = int(icol[t])
                if do_gather:
                    nc.gpsimd.dma_gather(
                        G[:, 0:chl, :], table[0:LOSPLIT, :],
                        idx_sb[:, ic:ic + chl * 8], chl * P, chl * P, F,
                        single_packet=False,
                    )
                    nc.gpsimd.dma_gather(
                        G[:, chl:ch, :], table[LOSPLIT:NPAD, :],
                        idx_sb[:, ic + chl * 8: ic + ch * 8], chh * P, chh * P, F,
                        single_packet=False,
                    )
                if not do_compute:
                    return None
                M = mp.tile([P, CH_MAX * P], f32, tag="M")
                oc = int(ocol[t])
                nc.vector.tensor_tensor(
                    out=M[:, : ch * P],
                    in0=off_sb[:, oc:oc + ch, None].to_broadcast([P, ch, P]),
                    in1=iota_sb[:, None, :].to_broadcast([P, ch, P]),
                    op=mybir.AluOpType.is_equal,
                )
                agg_ps = ps_agg.tile([F, P], f32, tag="agg")
                for k in range(ch):
                    nc.tensor.matmul(
                        out=agg_ps[:],
                        lhsT=G[:, k, :],
                        rhs=M[:, k * P:(k + 1) * P],
                        start=(k == 0),
                        stop=(k == ch - 1),
                    )
                agg_sb = sp.tile([F, P], f32, tag="agg_sb")
                nc.vector.tensor_copy(out=agg_sb[:], in_=agg_ps[:])
                return agg_sb

            # ---- layer 1 ----
            for _rep in range(repeat):
             for t in range(NT):
                agg_sb = aggregate(t, x_pad)
                if agg_sb is None:
                    continue
                h1a_ps = ps_mm.tile([H, P], f32, tag="mma")
                nc.tensor.matmul(out=h1a_ps[:], lhsT=w1a_sb[:], rhs=agg_sb[:],
                                 start=True, stop=True)
                h1a_sb = sp.tile([H, P], f32, tag="h1a")
                nc.scalar.activation(out=h1a_sb[:], in_=h1a_ps[:], func=Relu,
                                     bias=bnb_sb[:, :1], scale=bns_sb[:, :1])
                h1b_ps = ps_mm.tile([H, P], f32, tag="mmb")
                nc.tensor.matmul(out=h1b_ps[:], lhsT=w1b_sb[:], rhs=h1a_sb[:],
                                 start=True, stop=True)
                h1b_sb = sp.tile([H, P], f32, tag="h1b")
                nc.scalar.activation(out=h1b_sb[:], in_=h1b_ps[:], func=Relu)
                ht_ps = ps_agg.tile([P, H], f32, tag="trans")
                nc.tensor.transpose(out=ht_ps[:], in_=h1b_sb[:], identity=ident[:])
                ht_sb = sp.tile([P, H], f32, tag="ht")
                nc.vector.tensor_copy(out=ht_sb[:], in_=ht_ps[:])
                nc.sync.dma_start(out=h1_slice[t * P:(t + 1) * P, :], in_=ht_sb[:])

             # ---- all-gather h1 ----
             if do_cc:
              nc.gpsimd.collective_compute(
                "AllGather",
                mybir.AluOpType.bypass,
                replica_groups=[list(range(NCORES))],
                ins=[h1_slice.opt()],
                outs=[h1_full.opt()],
              )

             # ---- layer 2 ----
             for t in range(NT):
                agg_sb = aggregate(t, h1_full)
                if agg_sb is None:
                    continue
                h2_ps = ps_mm.tile([H, P], f32, tag="mma")
                nc.tensor.matmul(out=h2_ps[:], lhsT=w2a_sb[:], rhs=agg_sb[:],
                                 start=True, stop=True)
                h2_sb = sp.tile([H, P], f32, tag="h1a")
                nc.scalar.activation(out=h2_sb[:], in_=h2_ps[:], func=Relu)
                o_ps = ps_mm.tile([C, P], f32, tag="mmb")
                nc.tensor.matmul(out=o_ps[:], lhsT=w2b_sb[:], rhs=h2_sb[:],
                                 start=True, stop=True)
                o_sb = sp.tile([C, P], f32, tag="out")
                nc.scalar.activation(out=o_sb[:], in_=o_ps[:], func=Relu)
                nc.sync.dma_start(out=outT[:, t * P:(t + 1) * P], in_=o_sb[:])

    nc.compile()
    return nc


# ------------------------------------------------------------------- driver

_CACHE = {}


def kernel(x, edge_index, W1a, bn_gamma, bn_beta, bn_mean, bn_var, W1b, W2a, W2b,
           _trace=False):
    from concourse.bass_utils import run_bass_kernel_spmd

    in_maps, CH_LO, CH_HI, gid_of_orig = prepare(
        x, edge_index, W1a, bn_gamma, bn_beta, bn_mean, bn_var, W1b, W2a, W2b
    )
    key = (tuple(CH_LO), tuple(CH_HI))
    if key not in _CACHE:
        _CACHE[key] = build(CH_LO, CH_HI)
    nc = _CACHE[key]

    res = run_bass_kernel_spmd(nc, in_maps, core_ids=list(range(NCORES)))
    outT = np.concatenate([r["outT"] for r in res.results], axis=1)  # [C, NPAD]
    out = outT.T[gid_of_orig]  # [N, C]
    if _trace:
        kernel.last_results = res
    return np.ascontiguousarray(out.astype(np.float32))



# revision 4
# speedup vs baseline: 1.0216x; 1.0216x over previous
"""GIN message-passing kernel for 8 TRN2 NeuronCores.

Strategy: nodes sharded across 8 cores (6272 slots each, 50176 padded total).
Edges partitioned by destination node; source features gathered by row index
(dma_gather) from a replicated table (x for layer 1, AllGather'ed h1 for
layer 2). Per 128-node output tile, in-edges are packed into 128-edge chunks;
each chunk is gathered as G [128 edges, 128 feats] and accumulated into a
feature-major PSUM tile via agg[f, i] += G.T @ M, with M the one-hot
dst-offset matrix built on-device by a broadcast is_equal.

HW-bound facts (measured): dma_gather costs ~10ns per descriptor (per edge)
regardless of payload size/order — descriptor generation on 2 gpsimd cores
serializes. Hence the optimizations here all cut descriptor count or overlap:
  - self-edges (the +h term) are NOT gathered; each tile adds its own rows
    via an identity matmul on a sequentially-DMA'd tile.
  - chunk padding uses trailing -1 indices, which the gather skips; the
    per-gather valid count (uniform across cores by padding to the max) is
    passed as num_idxs_reg.
  - gathers alternate between 2 SWDGE queues (different gpsimd core pairs).
  - the h1 AllGather is split into 4 quarter collectives issued as layer-1
    quarters complete, overlapping the collective with layer-1 gathers.
dma_gather indices are int16, so gathers split at row 32768 (lo/hi base).
"""
import warnings

warnings.filterwarnings("ignore")

import numpy as np

N = 50000
E = 800000
F = 128
H = 128
C = 40
BN_EPS = 1e-5
NCORES = 8
P = 128
NPC = 6272           # node slots per core
NT = NPC // P        # 49 tiles per core
NPAD = NCORES * NPC  # 50176
LOSPLIT = 32768      # int16 gather index limit
QT = [0, 13, 25, 37, 49]   # quarter tile bounds for split AllGather
NQUARTER = len(QT) - 1
SPLIT_CC = True


# ----------------------------------------------------------------- host prep

def _assign_nodes(deg):
    """Greedy balanced assignment of nodes to (core, tile, slot).

    Returns gid_of_orig[N]: global slot id = c*NPC + t*P + s, balancing total
    degree per core and per tile so per-tile chunk counts are uniform.
    """
    order = np.argsort(-deg, kind="stable")
    core_load = np.zeros(NCORES, np.int64)
    core_cnt = np.zeros(NCORES, np.int64)
    node_core = np.empty(N, np.int32)
    for n in order:
        c = -1
        best = None
        for cc in range(NCORES):
            if core_cnt[cc] >= NPC:
                continue
            if best is None or core_load[cc] < best:
                best = core_load[cc]
                c = cc
        node_core[n] = c
        core_load[c] += deg[n]
        core_cnt[c] += 1

    gid_of_orig = np.empty(N, np.int64)
    for c in range(NCORES):
        nodes = order[node_core[order] == c]
        tile_load = np.zeros(NT, np.int64)
        tile_cnt = np.zeros(NT, np.int64)
        tl = np.empty(len(nodes), np.int32)
        # nodes are already degree-sorted desc; greedy least-loaded tile
        for i, n in enumerate(nodes):
            avail = tile_cnt < P
            t = np.where(avail, tile_load, np.iinfo(np.int64).max).argmin()
            tl[i] = t
            tile_load[t] += deg[n]
            tile_cnt[t] += 1
        slot = np.zeros(NT, np.int64)
        for i, n in enumerate(nodes):
            t = tl[i]
            gid_of_orig[n] = c * NPC + t * P + slot[t]
            slot[t] += 1
    return gid_of_orig


def _wrap_idx(idx):
    """[n] int -> [128, n//16] int16: idx i at [i%16, i//16], replicated x8."""
    n = len(idx)
    w = np.asarray(idx, np.int16).reshape(n // 16, 16).T
    return np.tile(w, (8, 1))


def _pack_edges(src_gid, dst_gid, rowmap, rows_total):
    """Partition edges by (core, tile), split lo/hi by gathered row id, sort
    by row, pad to the cross-core max valid count (V) with row 0 and then to
    a 128-multiple with idx -1 (skipped by the gather).

    rowmap: maps src gid -> row index in the gather table.
    Returns (CH_LO, CH_HI, V_LO, V_HI, idx_pack[c], off_pack[c]).
    """
    rows = rowmap[src_gid]
    core = dst_gid // NPC
    tile = (dst_gid % NPC) // P
    off = dst_gid % P
    is_lo = rows < LOSPLIT

    lists = [[None] * NT for _ in range(NCORES)]
    key = (core * NT + tile).astype(np.int64)
    order = np.argsort(key, kind="stable")
    rows_s, off_s, lo_s, key_s = rows[order], off[order], is_lo[order], key[order]
    bounds = np.searchsorted(key_s, np.arange(NCORES * NT + 1))
    for c in range(NCORES):
        for t in range(NT):
            b0, b1 = bounds[c * NT + t], bounds[c * NT + t + 1]
            m = lo_s[b0:b1]
            lr, lo_off = rows_s[b0:b1][m], off_s[b0:b1][m]
            hr, hi_off = rows_s[b0:b1][~m] - LOSPLIT, off_s[b0:b1][~m]
            so = np.argsort(lr, kind="stable")
            sh = np.argsort(hr, kind="stable")
            lists[c][t] = ((lr[so], lo_off[so]), (hr[sh], hi_off[sh]))

    V_LO = np.zeros(NT, np.int64)
    V_HI = np.zeros(NT, np.int64)
    for c in range(NCORES):
        for t in range(NT):
            lo, hi = lists[c][t]
            V_LO[t] = max(V_LO[t], len(lo[0]))
            V_HI[t] = max(V_HI[t], len(hi[0]))
    V_LO = np.maximum(V_LO, 16)
    V_HI = np.maximum(V_HI, 16)
    CH_LO = np.maximum(-(-V_LO // P), 1)
    CH_HI = np.maximum(-(-V_HI // P), 1)

    idx_pack, off_pack = [], []
    hi_rows = rows_total - LOSPLIT
    for c in range(NCORES):
        idx_cols, off_cols = [], []
        for t in range(NT):
            (lor, loo), (hir, hio) = lists[c][t]
            for r, o, V, CH, rmax in (
                (lor, loo, V_LO[t], CH_LO[t], LOSPLIT),
                (hir, hio, V_HI[t], CH_HI[t], hi_rows),
            ):
                nfull = CH * P
                ix = np.full(nfull, -1, np.int64)
                ix[: len(r)] = r
                ix[len(r): V] = 0          # valid pads up to uniform V
                fo = np.full(nfull, P, np.float32)
                fo[: len(o)] = o
                assert ix.max(initial=0) < rmax
                idx_cols.append(_wrap_idx(ix))
                # dstoff layout [128, CH]: chunk k, partition p = edge k*128+p
                off_cols.append(fo.reshape(CH, P).T.astype(np.float32))
        idx_pack.append(np.ascontiguousarray(np.concatenate(idx_cols, axis=1)))
        off_pack.append(np.ascontiguousarray(np.concatenate(off_cols, axis=1)))
    return CH_LO, CH_HI, V_LO, V_HI, idx_pack, off_pack


def _quarter_rowmap():
    """gid -> row in the quarter-interleaved h1_full layout produced by the
    4 partial AllGathers: quarter q holds rows [QBASE[q] + c*rq*P + local)."""
    rq = np.diff(np.asarray(QT)) * P              # rows per core per quarter
    qbase = np.concatenate([[0], np.cumsum(rq * NCORES)])[:-1]
    gid = np.arange(NPAD, dtype=np.int64)
    c = gid // NPC
    r = gid % NPC
    t = r // P
    q = np.searchsorted(np.asarray(QT[1:]) * P, r, side="right")
    return qbase[q] + c * rq[q] + (r - np.asarray(QT)[q] * P)


def prepare(x, edge_index, W1a, bn_gamma, bn_beta, bn_mean, bn_var, W1b, W2a, W2b):
    x = np.asarray(x, np.float32)
    ei = np.asarray(edge_index, np.int64)
    src_o, dst_o = ei[0], ei[1]

    deg = np.bincount(dst_o, minlength=N).astype(np.int64) + 1
    gid_of_orig = _assign_nodes(deg)

    # real edges only; the "+h" self term is added via an identity matmul
    src_gid = gid_of_orig[src_o]
    dst_gid = gid_of_orig[dst_o]

    ident_map = np.arange(NPAD, dtype=np.int64)
    pack1 = _pack_edges(src_gid, dst_gid, ident_map, NPAD)
    qmap = _quarter_rowmap()
    pack2 = _pack_edges(src_gid, dst_gid, qmap, NPAD)

    x_pad = np.zeros((NPAD, F), np.float32)
    x_pad[gid_of_orig] = x

    scale = (np.asarray(bn_gamma) / np.sqrt(np.asarray(bn_var) + BN_EPS)).astype(
        np.float32
    )
    bias = (np.asarray(bn_beta) - np.asarray(bn_mean) * scale).astype(np.float32)

    consts = {
        "x_pad": x_pad,
        "W1aT": np.ascontiguousarray(np.asarray(W1a, np.float32).T),
        "W1bT": np.ascontiguousarray(np.asarray(W1b, np.float32).T),
        "W2aT": np.ascontiguousarray(np.asarray(W2a, np.float32).T),
        "W2bT": np.ascontiguousarray(np.asarray(W2b, np.float32).T),
        "bn_s": scale.reshape(H, 1),
        "bn_b": bias.reshape(H, 1),
        "iota": np.tile(np.arange(P, dtype=np.float32), (P, 1)),
    }
    in_maps = []
    for c in range(NCORES):
        m = dict(consts)
        m["idx1"] = pack1[4][c]
        m["off1"] = pack1[5][c]
        m["idx2"] = pack2[4][c]
        m["off2"] = pack2[5][c]
        m["xloc"] = np.ascontiguousarray(x_pad[c * NPC:(c + 1) * NPC])
        in_maps.append(m)
    meta1 = pack1[:4]
    meta2 = pack2[:4]
    return in_maps, meta1, meta2, gid_of_orig


# -------------------------------------------------------------- bass program

def build(meta1, meta2, do_gather=True, do_compute=True, do_cc=True,
          split_cc=SPLIT_CC):
    import concourse.bacc as bacc
    import concourse.mybir as mybir
    import concourse.tile as tile
    from concourse.masks import make_identity

    nc = bacc.Bacc("TRN2", target_bir_lowering=False, debug=False,
                   num_devices=NCORES, num_swdge_queues=2)
    f32 = mybir.dt.float32

    def stot(meta):
        return int(8 * (meta[0].sum() + meta[1].sum()))

    def ctot(meta):
        return int(meta[0].sum() + meta[1].sum())

    CH_MAX = int(max((meta1[0] + meta1[1]).max(), (meta2[0] + meta2[1]).max()))

    x_pad = nc.dram_tensor("x_pad", [NPAD, F], f32, kind="ExternalInput")
    xloc = nc.dram_tensor("xloc", [NPC, F], f32, kind="ExternalInput")
    idx1 = nc.dram_tensor("idx1", [P, stot(meta1)], mybir.dt.int16,
                          kind="ExternalInput")
    off1 = nc.dram_tensor("off1", [P, ctot(meta1)], f32, kind="ExternalInput")
    idx2 = nc.dram_tensor("idx2", [P, stot(meta2)], mybir.dt.int16,
                          kind="ExternalInput")
    off2 = nc.dram_tensor("off2", [P, ctot(meta2)], f32, kind="ExternalInput")
    W1aT = nc.dram_tensor("W1aT", [F, H], f32, kind="ExternalInput")
    W1bT = nc.dram_tensor("W1bT", [H, H], f32, kind="ExternalInput")
    W2aT = nc.dram_tensor("W2aT", [H, H], f32, kind="ExternalInput")
    W2bT = nc.dram_tensor("W2bT", [H, C], f32, kind="ExternalInput")
    bn_s = nc.dram_tensor("bn_s", [H, 1], f32, kind="ExternalInput")
    bn_b = nc.dram_tensor("bn_b", [H, 1], f32, kind="ExternalInput")
    iota = nc.dram_tensor("iota", [P, P], f32, kind="ExternalInput")
    outT = nc.dram_tensor("outT", [C, NPC], f32, kind="ExternalOutput")

    Relu = mybir.ActivationFunctionType.Relu
    qcount = [0]

    with tile.TileContext(nc) as tc:
        with (
            tc.tile_pool(name="const", bufs=1) as cst,
            tc.tile_pool(name="gbuf", bufs=3) as gp,
            tc.tile_pool(name="mbuf", bufs=3) as mp,
            tc.tile_pool(name="small", bufs=3) as sp,
            tc.tile_pool(name="selfb", bufs=3) as selfp,
            tc.tile_pool(name="ps_agg", bufs=2, space="PSUM") as ps_agg,
            tc.tile_pool(name="ps_mm", bufs=2, space="PSUM") as ps_mm,
            tc.tile_pool(name="dram", bufs=1, space="DRAM") as dram,
        ):
            ident = cst.tile([P, P], f32)
            make_identity(nc, ident[:])
            iota_sb = cst.tile([P, P], f32)
            nc.sync.dma_start(out=iota_sb[:], in_=iota[:])
            w1a_sb = cst.tile([F, H], f32)
            nc.sync.dma_start(out=w1a_sb[:], in_=W1aT[:])
            w1b_sb = cst.tile([H, H], f32)
            nc.sync.dma_start(out=w1b_sb[:], in_=W1bT[:])
            w2a_sb = cst.tile([H, H], f32)
            nc.sync.dma_start(out=w2a_sb[:], in_=W2aT[:])
            w2b_sb = cst.tile([H, C], f32)
            nc.sync.dma_start(out=w2b_sb[:], in_=W2bT[:])
            bns_sb = cst.tile([H, 1], f32)
            nc.sync.dma_start(out=bns_sb[:], in_=bn_s[:])
            bnb_sb = cst.tile([H, 1], f32)
            nc.sync.dma_start(out=bnb_sb[:], in_=bn_b[:])
            idx1_sb = cst.tile([P, stot(meta1)], mybir.dt.int16)
            nc.sync.dma_start(out=idx1_sb[:], in_=idx1[:])
            off1_sb = cst.tile([P, ctot(meta1)], f32)
            nc.sync.dma_start(out=off1_sb[:], in_=off1[:])
            idx2_sb = cst.tile([P, stot(meta2)], mybir.dt.int16)
            nc.sync.dma_start(out=idx2_sb[:], in_=idx2[:])
            off2_sb = cst.tile([P, ctot(meta2)], f32)
            nc.sync.dma_start(out=off2_sb[:], in_=off2[:])

            h1_slice = dram.tile([NPC, H], f32)
            h1_full = dram.tile([NPAD, H], f32)
            rq = [(QT[i + 1] - QT[i]) * P for i in range(NQUARTER)]
            qbase = np.concatenate([[0], np.cumsum([r * NCORES for r in rq])])

            def cols(meta):
                icol = np.concatenate(
                    [[0], np.cumsum((meta[0] + meta[1]) * 8)])
                ocol = np.concatenate([[0], np.cumsum(meta[0] + meta[1])])
                return icol, ocol

            icol1, ocol1 = cols(meta1)
            icol2, ocol2 = cols(meta2)

            def aggregate(t, table, meta, icol, ocol, idx_sb, off_sb, self_src):
                """Gather + segment-sum for tile t -> PSUM [F, P] feat-major.

                self_src: dram AP of this tile's own rows (identity chunk)."""
                CHL, CHH, VL, VH = meta
                chl, chh = int(CHL[t]), int(CHH[t])
                ch = chl + chh
                agg_ps = ps_agg.tile([F, P], f32, tag="agg")
                xt = selfp.tile([P, F], f32, tag="xt")
                nc.sync.dma_start(out=xt[:], in_=self_src)
                nc.tensor.matmul(out=agg_ps[:], lhsT=xt[:], rhs=ident[:],
                                 start=True, stop=False)
                G = gp.tile([P, CH_MAX, F], f32, tag="G")
                ic = int(icol[t])
                if do_gather:
                    nc.gpsimd.dma_gather(
                        G[:, 0:chl, :], table[0:LOSPLIT, :],
                        idx_sb[:, ic:ic + chl * 8], chl * P, int(VL[t]), F,
                        single_packet=False, queue_num=qcount[0] % 2,
                    )
                    nc.gpsimd.dma_gather(
                        G[:, chl:ch, :], table[LOSPLIT:NPAD, :],
                        idx_sb[:, ic + chl * 8: ic + ch * 8], chh * P,
                        int(VH[t]), F,
                        single_packet=False, queue_num=(qcount[0] + 1) % 2,
                    )
                    qcount[0] += 1
                if not do_compute:
                    nc.tensor.matmul(out=agg_ps[:], lhsT=xt[:], rhs=ident[:],
                                     start=False, stop=True)
                    return None
                M = mp.tile([P, CH_MAX * P], f32, tag="M")
                oc = int(ocol[t])
                nc.vector.tensor_tensor(
                    out=M[:, : ch * P],
                    in0=off_sb[:, oc:oc + ch, None].to_broadcast([P, ch, P]),
                    in1=iota_sb[:, None, :].to_broadcast([P, ch, P]),
                    op=mybir.AluOpType.is_equal,
                )
                for k in range(ch):
                    nc.tensor.matmul(
                        out=agg_ps[:],
                        lhsT=G[:, k, :],
                        rhs=M[:, k * P:(k + 1) * P],
                        start=False,
                        stop=(k == ch - 1),
                    )
                agg_sb = sp.tile([F, P], f32, tag="agg_sb")
                nc.vector.tensor_copy(out=agg_sb[:], in_=agg_ps[:])
                return agg_sb

            # ---- layer 1 ----
            qnext = 0
            for t in range(NT):
                agg_sb = aggregate(t, x_pad, meta1, icol1, ocol1, idx1_sb,
                                   off1_sb, xloc[t * P:(t + 1) * P, :])
                if agg_sb is not None:
                    h1a_ps = ps_mm.tile([H, P], f32, tag="mma")
                    nc.tensor.matmul(out=h1a_ps[:], lhsT=w1a_sb[:], rhs=agg_sb[:],
                                     start=True, stop=True)
                    h1a_sb = sp.tile([H, P], f32, tag="h1a")
                    nc.scalar.activation(out=h1a_sb[:], in_=h1a_ps[:], func=Relu,
                                         bias=bnb_sb[:, :1], scale=bns_sb[:, :1])
                    h1b_ps = ps_mm.tile([H, P], f32, tag="mmb")
                    nc.tensor.matmul(out=h1b_ps[:], lhsT=w1b_sb[:], rhs=h1a_sb[:],
                                     start=True, stop=True)
                    h1b_sb = sp.tile([H, P], f32, tag="h1b")
                    nc.scalar.activation(out=h1b_sb[:], in_=h1b_ps[:], func=Relu)
                    ht_ps = ps_agg.tile([P, H], f32, tag="trans")
                    nc.tensor.transpose(out=ht_ps[:], in_=h1b_sb[:],
                                        identity=ident[:])
                    ht_sb = sp.tile([P, H], f32, tag="ht")
                    nc.vector.tensor_copy(out=ht_sb[:], in_=ht_ps[:])
                    nc.sync.dma_start(out=h1_slice[t * P:(t + 1) * P, :],
                                      in_=ht_sb[:])
                if do_cc and split_cc and t == QT[qnext + 1] - 1:
                    q = qnext
                    nc.gpsimd.collective_compute(
                        "AllGather",
                        mybir.AluOpType.bypass,
                        replica_groups=[list(range(NCORES))],
                        ins=[h1_slice[QT[q] * P:QT[q + 1] * P, :]],
                        outs=[h1_full[int(qbase[q]):int(qbase[q + 1]), :]],
                    )
                    qnext += 1

            if do_cc and not split_cc:
                nc.gpsimd.collective_compute(
                    "AllGather",
                    mybir.AluOpType.bypass,
                    replica_groups=[list(range(NCORES))],
                    ins=[h1_slice.opt()],
                    outs=[h1_full.opt()],
                )

            # ---- layer 2 ----
            # h1 rows of this core live at h1_full[qbase[q] + my quarter rows)
            # when split_cc; read the local copy from h1_slice instead.
            meta2_eff = meta2 if split_cc or not do_cc else meta2
            for t in range(NT):
                agg_sb = aggregate(t, h1_full, meta2_eff, icol2, ocol2, idx2_sb,
                                   off2_sb, h1_slice[t * P:(t + 1) * P, :])
                if agg_sb is None:
                    continue
                h2_ps = ps_mm.tile([H, P], f32, tag="mma")
                nc.tensor.matmul(out=h2_ps[:], lhsT=w2a_sb[:], rhs=agg_sb[:],
                                 start=True, stop=True)
                h2_sb = sp.tile([H, P], f32, tag="h1a")
                nc.scalar.activation(out=h2_sb[:], in_=h2_ps[:], func=Relu)
                o_ps = ps_mm.tile([C, P], f32, tag="mmb")
                nc.tensor.matmul(out=o_ps[:], lhsT=w2b_sb[:], rhs=h2_sb[:],
                                 start=True, stop=True)
                o_sb = sp.tile([C, P], f32, tag="out")
                nc.scalar.activation(out=o_sb[:], in_=o_ps[:], func=Relu)
                nc.sync.dma_start(out=outT[:, t * P:(t + 1) * P], in_=o_sb[:])

    nc.compile()
    return nc


# ------------------------------------------------------------------- driver

_CACHE = {}


def build_cached(x, edge_index, W1a, bn_gamma, bn_beta, bn_mean, bn_var, W1b,
                 W2a, W2b, **build_kw):
    in_maps, meta1, meta2, gid_of_orig = prepare(
        x, edge_index, W1a, bn_gamma, bn_beta, bn_mean, bn_var, W1b, W2a, W2b
    )
    key = (tuple(meta1[0]), tuple(meta1[1]), tuple(meta2[0]), tuple(meta2[1]),
           tuple(sorted(build_kw.items())))
    if key not in _CACHE:
        _CACHE[key] = build(meta1, meta2, **build_kw)
    return _CACHE[key], in_maps, gid_of_orig


def kernel(x, edge_index, W1a, bn_gamma, bn_beta, bn_mean, bn_var, W1b, W2a, W2b):
    from concourse.bass_utils import run_bass_kernel_spmd

    nc, in_maps, gid_of_orig = build_cached(
        x, edge_index, W1a, bn_gamma, bn_beta, bn_mean, bn_var, W1b, W2a, W2b
    )
    res = run_bass_kernel_spmd(nc, in_maps, core_ids=list(range(NCORES)))
    outT = np.concatenate([r["outT"] for r in res.results], axis=1)  # [C, NPAD]
    out = outT.T[gid_of_orig]  # [N, C]
    return np.ascontiguousarray(out.astype(np.float32))
